# revision 2
# baseline (speedup 1.0000x reference)
"""LSTM encoder (T=512, B=256, H=256, V=32000) on 8 trn2 NeuronCores.

Strategy
--------
Data-parallel over batch: B=256 -> 32 per core; weights/table replicated.

The recurrence is latency-chain bound (512 serial steps); the kernel
minimizes the per-step dependency cycle:

  burst(g,f,i) -> sigma(g,f,i) -> u=f*c~ ; t=(s_g-0.5)*i ; c~=u+t
  -> s_c=sigma(4*c~) -> h~=(s_c-0.5)*o -> next burst

All-sigmoid trick: tanh(x) = 2*sigmoid(2x) - 1 folded into pre-scaled
weights, so ONE activation covers g,f,i (192 contiguous elements). The
cell state is stored as c~ = c/2 and the hidden as h~ = h/2 with the
x2 compensated inside W_hh (exact powers of two, no precision loss):

  W_hh' = 2*W_hh (g rows x4), W_ih' g rows x2, b' g rows x2
  c~' = f*c~ + i*(sigma(2zg) - 0.5)        [TT + STT + TT on DVE]
  h~  = (sigma(4c~') - 0.5) * o            [one fused STT on DVE]

PSUM is laid out s-major ([128, S, chunks, BL]) so every per-step
activation reads ONE contiguous descriptor: measured ACT cost drops
from 366ns (4-desc strided) to ~130-240ns (contig). Gate chunk order is
[g, f, i, o]: the g,f,i gates live in one 3-bank PSUM tile read by a
single sigma; the o gate has its own bank (sigma(o) runs in ACT shadow).

Bias is materialized by indicator matmuls (lhsT = per-chunk bias rows,
rhs = 0/1 indicator matrix) that also perform the start=True bank init;
x-projections W_ih @ emb for a window of S=8 steps accumulate behind
each step's recurrent burst, filling the PE idle tail.

Embeddings are fetched with dma_gather(transpose=True): fp16 table rows
deposited H-on-partitions, the exact rhs layout the x-matmuls need.

Numerics: fp16 table/weights/h~ (matmul operands), fp32 PSUM and fp32
elementwise state c~. Host post-scales outputs by 2 (exact).
"""

import numpy as np

T, B, H, V = 512, 256, 256, 32000
N_CORES = 8
BL = B // N_CORES          # 32 batch per core
S = 8                      # steps per PSUM window
G4 = 4 * H                 # 1024
M = G4 // 128              # 8 gate chunks
K = H // 128               # 2 contraction chunks
MGFI = 3 * K               # 6 chunks for g,f,i
FLAT = S * MGFI * BL       # 1536 flat elements of the pgfi tile

# gate chunk order g, f, i, o (PyTorch native is i, f, g, o)
_PERM = np.concatenate([
    np.arange(2 * H, 3 * H),   # g
    np.arange(H, 2 * H),       # f
    np.arange(0, H),           # i
    np.arange(3 * H, 4 * H),   # o
])


def _build_bass(t_steps=T, keepwarm=True):
    from contextlib import ExitStack
    from concourse import bacc, mybir, library_config
    import concourse.tile as tile

    f16, f32, i16 = mybir.dt.float16, mybir.dt.float32, mybir.dt.int16
    Sig = mybir.ActivationFunctionType.Sigmoid
    mult = mybir.AluOpType.mult
    add = mybir.AluOpType.add
    sub = mybir.AluOpType.subtract

    NW = t_steps // S
    NI = S * BL            # 256 gathered rows per window, t-major

    nc = bacc.Bacc("TRN2", target_bir_lowering=False, debug=False)
    idx_d = nc.declare_dram_parameter("idx", [128, NW, NI // 16], i16, isOutput=False)
    tab_d = nc.declare_dram_parameter("table", [V, H], f16, isOutput=False)
    wih_d = nc.declare_dram_parameter("wih_t", [H, G4], f16, isOutput=False)
    whh_d = nc.declare_dram_parameter("whh_t", [H, G4], f16, isOutput=False)
    b_d = nc.declare_dram_parameter("bias", [M, 128], f16, isOutput=False)
    ind_d = nc.declare_dram_parameter("ind", [MGFI, FLAT + 512], f16, isOutput=False)
    h0_d = nc.declare_dram_parameter("h0t", [128, K, BL], f16, isOutput=False)
    c0_d = nc.declare_dram_parameter("c0t", [128, K, BL], f32, isOutput=False)
    ho_d = nc.declare_dram_parameter("h_out", [128, K, BL], f32, isOutput=True)
    co_d = nc.declare_dram_parameter("c_out", [128, K, BL], f32, isOutput=True)

    import bass_rust

    with tile.TileContext(nc) as tc, ExitStack() as ctx:
        const = ctx.enter_context(tc.tile_pool(name="const", bufs=1))
        embp = ctx.enter_context(tc.tile_pool(name="embp", bufs=3))
        psum = ctx.enter_context(tc.tile_pool(name="psum", bufs=2, space="PSUM"))
        sp = ctx.enter_context(tc.tile_pool(name="sp", bufs=3))
        tmp = ctx.enter_context(tc.tile_pool(name="tmp", bufs=3))
        hp = ctx.enter_context(tc.tile_pool(name="hp", bufs=3))

        # idx upload + library load first so the first gather's Q7 work
        # overlaps the remaining constant DMAs
        idx_sb = const.tile([128, NW, NI // 16], i16, name="idx_sb")
        nc.sync.dma_start(idx_sb[:], idx_d[:])
        nc.gpsimd.load_library(library_config.mlp)
        whh_sb, wih_sb = [], []
        for k in range(K):
            wt = const.tile([128, G4], f16, name=f"whh_sb{k}")
            nc.sync.dma_start(wt[:], whh_d[128 * k:128 * (k + 1), :])
            whh_sb.append(wt)
            xt = const.tile([128, G4], f16, name=f"wih_sb{k}")
            nc.sync.dma_start(xt[:], wih_d[128 * k:128 * (k + 1), :])
            wih_sb.append(xt)
        b6_sb = const.tile([MGFI, 128], f16, name="b6_sb")
        nc.sync.dma_start(b6_sb[:], b_d[0:MGFI, :])
        b2_sb = const.tile([K, 128], f16, name="b2_sb")
        nc.sync.dma_start(b2_sb[:], b_d[MGFI:M, :])
        ind = const.tile([MGFI, FLAT + 512], f16, name="ind")
        nc.sync.dma_start(ind[:], ind_d[:])

        # state: c~ (fp32), h~ (fp16)
        ct = const.tile([128, K, BL], f32, name="ct")
        nc.sync.dma_start(ct[:], c0_d[:])
        h_cur = const.tile([128, K, BL], f16, name="h0_sb")
        nc.sync.dma_start(h_cur[:], h0_d[:])

        embt = {}
        ps = {}

        def gather(w):
            e = embp.tile([128, K, NI], f16, name="embt", tag=f"embt{w % 3}",
                          bufs=1)
            g_i = nc.gpsimd.dma_gather(
                out_ap=e[:], in_ap=tab_d[:],
                idxs_ap=idx_sb[:, w, :],
                num_idxs=NI, num_idxs_reg=NI, elem_size=H, transpose=True)
            embt[w] = e
            return g_i

        def alloc_ps(w):
            # s-major PSUM: every per-step activation reads one contiguous
            # strip. pgfi = g,f,i chunks (3 banks), po = o chunks (1 bank).
            pgfi = psum.tile([128, S, MGFI, BL], f32, name="pgfi",
                             tag=f"pgfi{w % 2}", bufs=1)
            po = psum.tile([128, S, K, BL], f32, name="po",
                           tag=f"po{w % 2}", bufs=1)
            ps[w] = (pgfi, po)

        def bias_mms(w, lo, hi, after=None):
            # first write to each bank: N=512 indicator matmul, start=True.
            # jobs 0..2 cover pgfi's 3 banks, job 3 covers po's bank.
            if lo == 0 and w not in ps:
                alloc_ps(w)
            pgfi, po = ps[w]
            flat = pgfi[:].rearrange("p a b c -> p (a b c)")
            for b in range(lo, hi):
                if b < 3:
                    mm = nc.tensor.matmul(
                        out=flat[:, 512 * b:512 * (b + 1)],
                        lhsT=b6_sb[:],
                        rhs=ind[:, 512 * b:512 * (b + 1)],
                        start=True, stop=False, skip_group_check=True)
                else:
                    mm = nc.tensor.matmul(
                        out=po[:].rearrange("p a b c -> p (a b c)"),
                        lhsT=b2_sb[:],
                        rhs=ind[0:K, FLAT:FLAT + 512],
                        start=True, stop=False, skip_group_check=True)
                if after is not None:
                    bass_rust.add_dep_helper(mm.ins, after.ins, sync=False,
                                             reason="pin bias after burst")

        def x_mms(w, lo, hi, after=None):
            pgfi, po = ps[w]
            for j in range(lo, hi):
                m, k = j // K, j % K
                out = pgfi[:, :, m, :] if m < MGFI else po[:, :, m - MGFI, :]
                mm = nc.tensor.matmul(
                    out=out,
                    lhsT=wih_sb[k][:, 128 * m:128 * (m + 1)],
                    rhs=embt[w][:, k, :],
                    start=False, stop=False, skip_group_check=True)
                if after is not None:
                    bass_rust.add_dep_helper(mm.ins, after.ins, sync=False,
                                             reason="pin x after burst")

        def burst(w, s, mlo, mhi):
            pgfi, po = ps[w]
            last = None
            for m in range(mlo, mhi):
                out = pgfi[:, s, m, :] if m < MGFI else po[:, s, m - MGFI, :]
                for k in range(K):
                    last = nc.tensor.matmul(
                        out=out,
                        lhsT=whh_sb[k][:, 128 * m:128 * (m + 1)],
                        rhs=h_cur[:, k, :],
                        start=False, stop=(k == K - 1), skip_group_check=True)
            return last

        # prologue: window 0 fully prepared, window 1 gathered
        gather(0)
        if NW > 1:
            gather(1)
        # PE p-state warm-up: sustained matmuls trigger the 1.2 -> 2.4 GHz
        # clock ramp. Results are garbage, overwritten by start=True bias.
        alloc_ps(0)
        warm_flat = ps[0][0][:].rearrange("p a b c -> p (a b c)")
        for _ in range(35):
            nc.tensor.matmul(
                out=warm_flat[:, 0:512], lhsT=whh_sb[0][:, 0:128],
                rhs=whh_sb[0][:, 0:512], start=True, stop=True,
                skip_group_check=True)
        bias_mms(0, 0, 4)
        x_mms(0, 0, M * K)

        for w in range(NW):
            pgfi_w, po_w = ps[w]
            for s in range(S):
                t = w * S + s
                burst(w, s, 0, MGFI)                 # g, f, i chunks
                sall = sp.tile([128, M, BL], f16, name="sall", tag="sall")
                u32 = tmp.tile([128, K, BL], f32, name="u32", tag="u32")
                t16 = tmp.tile([128, K, BL], f16, name="t16", tag="t16")
                sc16 = tmp.tile([128, K, BL], f16, name="sc16", tag="sc16")
                nc.scalar.activation(sall[:, 0:MGFI, :],
                                     pgfi_w[:, s, :, :], Sig)
                last_mm = burst(w, s, MGFI, M)       # o chunks
                nc.scalar.activation(sall[:, MGFI:M, :],
                                     po_w[:, s, :, :], Sig)
                # DVE tail: u = f*c~ ; t = (s_g-0.5)*i ; c~ = u + t
                nc.vector.tensor_tensor(
                    out=u32[:], in0=sall[:, 2:4, :], in1=ct[:], op=mult)
                nc.vector.scalar_tensor_tensor(
                    out=t16[:], in0=sall[:, 0:2, :], scalar=0.5,
                    in1=sall[:, 4:6, :], op0=sub, op1=mult)
                nc.vector.tensor_tensor(
                    out=ct[:], in0=u32[:], in1=t16[:], op=add)
                nc.scalar.activation(sc16[:], ct[:], Sig, scale=4.0)
                if t < t_steps - 1:
                    hn = hp.tile([128, K, BL], f16, name="hn", tag="hn")
                    nc.vector.scalar_tensor_tensor(
                        out=hn[:], in0=sc16[:], scalar=0.5,
                        in1=sall[:, 6:8, :], op0=sub, op1=mult)
                    h_cur = hn
                else:
                    hf = tmp.tile([128, K, BL], f32, name="hf", tag="hf")
                    nc.vector.scalar_tensor_tensor(
                        out=hf[:], in0=sc16[:], scalar=0.5,
                        in1=sall[:, 6:8, :], op0=sub, op1=mult)
                    nc.sync.dma_start(ho_d[:], hf[:])
                    nc.sync.dma_start(co_d[:], ct[:])
                # window w+1 prep + window w+2 gather spread across steps,
                # pinned behind this step's burst to fill the PE idle tail.
                if w + 1 < NW:
                    if s == 0:
                        if w + 2 < NW:
                            gather(w + 2)
                        bias_mms(w + 1, 0, 2, after=last_mm)
                    elif s == 1:
                        bias_mms(w + 1, 2, 4, after=last_mm)
                    elif s <= 6:
                        n_x = M * K
                        lo = (s - 2) * n_x // 5
                        hi = (s - 1) * n_x // 5
                        x_mms(w + 1, lo, hi, after=last_mm)
                # PE keep-warm: garbage matmuls into this window's already
                # consumed s-strips (re-initialized by w+2's start=True
                # bias). Holds the 2.4 GHz p-state through per-step gaps.
                if keepwarm and s >= 2:
                    n_str = min(s - 1, 2)
                    for j in range(3):
                        dm = nc.tensor.matmul(
                            out=pgfi_w[:, 0:n_str, 2 * (j % 3):2 * (j % 3) + 2, :],
                            lhsT=whh_sb[0][:, 0:128],
                            rhs=whh_sb[0][:, 0:n_str * 2 * BL],
                            start=False, stop=False, skip_group_check=True)
                        bass_rust.add_dep_helper(
                            dm.ins, last_mm.ins, sync=False,
                            reason="pin keep-warm after burst")
            if w > 0:
                ps.pop(w - 1, None)
                embt.pop(w - 1, None)
    nc.finalize()
    return nc


def _prep_inputs(enc_inputs, h0, c0, embed, W_ih, W_hh, b_ih, b_hh, t_steps=T):
    """Host-side shard + layout prep. Returns list of per-core in_maps."""
    Wih_p = W_ih[_PERM].astype(np.float32).copy()
    Whh_p = W_hh[_PERM].astype(np.float32) * 2.0   # h~ = h/2 compensation
    b_p = (b_ih + b_hh)[_PERM].astype(np.float32).copy()
    Wih_p[0:H] *= 2.0                              # g rows: tanh = 2*sig(2x)-1
    Whh_p[0:H] *= 2.0
    b_p[0:H] *= 2.0
    wih_t = np.ascontiguousarray(Wih_p.T).astype(np.float16)   # [H, 4H]
    whh_t = np.ascontiguousarray(Whh_p.T).astype(np.float16)
    bias = np.ascontiguousarray(b_p.astype(np.float16).reshape(M, 128))
    table = embed.astype(np.float16)                           # [V, H]
    NI = S * BL
    # indicator matrix for the bias matmuls: position j of the flattened
    # pgfi tile holds chunk m(j) = (j % (MGFI*BL)) // BL; po: (j%64)//32
    ind = np.zeros((MGFI, FLAT + 512), np.float16)
    j = np.arange(FLAT)
    ind[(j % (MGFI * BL)) // BL, j] = 1.0
    j2 = np.arange(512)
    ind[(j2 % (K * BL)) // BL, FLAT + j2] = 1.0

    NW = t_steps // S
    in_maps = []
    for c in range(N_CORES):
        wrapped = np.empty((128, NW, NI // 16), np.int16)
        for w in range(NW):
            bs = slice(c * BL, (c + 1) * BL)
            flat = enc_inputs[w * S:(w + 1) * S, bs].astype(np.int16).reshape(-1)
            w16 = flat.reshape(-1, 16).T                       # [16, 16]
            wrapped[:, w, :] = np.tile(w16, (8, 1))
        bs = slice(c * BL, (c + 1) * BL)
        h0t = np.empty((128, K, BL), np.float16)
        c0t = np.empty((128, K, BL), np.float32)
        for k in range(K):
            h0t[:, k, :] = h0[bs].T[128 * k:128 * (k + 1), :] * 0.5
            c0t[:, k, :] = c0[bs].T[128 * k:128 * (k + 1), :] * 0.5
        in_maps.append({
            "idx": np.ascontiguousarray(wrapped), "table": table,
            "wih_t": wih_t, "whh_t": whh_t,
            "bias": bias, "ind": ind, "h0t": h0t, "c0t": c0t,
        })
    return in_maps


def _unshard(results):
    h = np.empty((B, H), np.float32)
    c = np.empty((B, H), np.float32)
    for core, out in enumerate(results):
        bs = slice(core * BL, (core + 1) * BL)
        for k in range(K):
            h[bs, 128 * k:128 * (k + 1)] = out["h_out"][:, k, :].T * 2.0
            c[bs, 128 * k:128 * (k + 1)] = out["c_out"][:, k, :].T * 2.0
    return h, c


def kernel(enc_inputs, h0, c0, embed, W_ih, W_hh, b_ih, b_hh):
    from concourse.bass_utils import run_bass_kernel_spmd

    enc_inputs = np.asarray(enc_inputs)
    h0 = np.asarray(h0, dtype=np.float32)
    c0 = np.asarray(c0, dtype=np.float32)
    embed = np.asarray(embed, dtype=np.float32)
    W_ih = np.asarray(W_ih, dtype=np.float32)
    W_hh = np.asarray(W_hh, dtype=np.float32)
    b_ih = np.asarray(b_ih, dtype=np.float32)
    b_hh = np.asarray(b_hh, dtype=np.float32)

    nc = _build_bass()
    in_maps = _prep_inputs(enc_inputs, h0, c0, embed, W_ih, W_hh, b_ih, b_hh)
    res = run_bass_kernel_spmd(nc, in_maps, core_ids=list(range(N_CORES)))
    return _unshard(res.results)
